# revision 5
# baseline (speedup 1.0000x reference)
"""DeltaDequantization Trainium2 kernel v8 (8-core SPMD, data parallel over batch).

Math (per batch element b, chunks c of 32 steps):
    scale_c = (1/32) * sum_{t in c, n} x[b,t,n] * cs[n]      (independent of carry)
    S_c     = prod_{c'<c} scale_c'            (exclusive cumprod)
    y[b,t]  = sum_n x[b,t,n] * qb[n]
    m_c     = (1/32) * sum_{t in c} y[b,t]
    pred_c  = sum_{c'<c} S_c' * m_c'          (exclusive cumsum)
    out[b,t]= pred_c(t) + S_c(t) * y[b,t]

Design: x is pre-cast to fp16 and pre-transposed on the HOST into
(t'',n)-on-partition layout; each 4-timestep block is a 128x128 fp16 PE
stationary (contraction over p = 32*t''+n), the moving operand is a fixed
6-column matrix A6 = [delta(t'')*qb[n] x4, cs[n]/32, qb[n]/32] marshalled
on the host.  Output lands BATCH-major [b, y0..y3, w, m] in PSUM - no
on-device transposes or back path.  DVE does per-span (4 strips = 8
chunks) clusters: ONE merged w/m reduce (permuted 5D view, block dim
innermost) + (cumprod S, tau=S*m, cumsum pred) + 2 affine ops reading y
straight from PSUM, writing bf16 out.  All 8 PSUM banks are span tiles.

Schedule: all x loads are issued up front on the ACT HWDGE ring (8 DMA
lanes pace them; ring saturated at ~330 GB/s, the HBM/NC roofline).
Loads are 7 x 2 MiB quads + 4 x 512 KiB strips for the last span so the
tail processes half-spans as data dribbles in.  Stores are per-(half)span
bf16 on the sync ring.  HBM traffic per core: 16.8 MiB read + 0.5 MiB
write (f32 baseline: 33.5 MiB).
"""

import numpy as np

import concourse.bass as bass
import concourse.bacc as bacc
import concourse.tile as tile
from concourse import mybir
from concourse.bass_utils import run_bass_kernel_spmd

F32 = mybir.dt.float32
F16 = mybir.dt.float16
BF16 = mybir.dt.bfloat16

B, T, NB = 1024, 2048, 32
NCORES = 8
BS = B // NCORES          # 128 batch rows per core = full partition dim
ADAPT = 32
C = T // ADAPT            # 64 chunks
STRIP_T = 64              # timesteps per strip
NSTRIP = T // STRIP_T     # 32
SF = STRIP_T * NB         # 2048 elements per partition per strip
NQUAD = NSTRIP // 4       # 8 quad-loads (4 strips = 1 span = 2 MiB each)

_cached_nc = None


def build_kernel():
    nc = bacc.Bacc("TRN2", target_bir_lowering=False, debug=False)

    # host-pretransposed x: row = 128*quad + (32*t'' + n),
    # col = sq*2048 + blk*128 + b   (strip s = 4*quad + sq, t = 64*s + 4*blk + t'')
    x_ext = nc.dram_tensor("x", [NQUAD * 128, 4 * SF], F16, kind="ExternalInput")
    # A6[p=(t'',n), :] = [delta(t'',0..3)*qb[n], cs[n]/32, qb[n]/32]
    a6_ext = nc.dram_tensor("a6", [128, 6], F16, kind="ExternalInput")
    out_ext = nc.dram_tensor("out", [BS, T], BF16, kind="ExternalOutput")

    ADD = mybir.AluOpType.add
    MUL = mybir.AluOpType.mult
    BYP = mybir.AluOpType.bypass

    with tile.TileContext(nc) as tc:
        with (
            tc.tile_pool(name="consts", bufs=1) as consts,
            tc.tile_pool(name="xpool", bufs=NQUAD) as xpool,
            tc.tile_pool(name="accpool", bufs=1) as accpool,
            tc.tile_pool(name="pspool", bufs=8, space="PSUM") as pspool,
        ):
            S_chain = consts.tile([128, C + 1], F32)
            pred_chain = consts.tile([128, C + 1], F32)
            pm_buf = consts.tile([128, 2 * C], F32)
            tau_buf = consts.tile([128, C], F32)
            A6 = consts.tile([128, 6], F16)
            out_sb = accpool.tile([128, T], BF16)

            xh = [
                xpool.tile([128, 4 * SF], F16, name="xh", tag="xh")
                for _ in range(NQUAD)
            ]

            # all x loads issued up front on the ACT HWDGE ring; the 8 DMA
            # lanes pace them.  Last quad split per strip for a short tail.
            for q in range(NQUAD - 1):
                nc.scalar.dma_start(
                    out=xh[q][:], in_=x_ext[q * 128:(q + 1) * 128, :]
                )
            lq = NQUAD - 1
            for sq in range(4):
                nc.scalar.dma_start(
                    out=xh[lq][:, sq * SF:(sq + 1) * SF],
                    in_=x_ext[lq * 128:(lq + 1) * 128, sq * SF:(sq + 1) * SF],
                )

            # A6 via sync ring (ahead of the stores); scan chain seeds on DVE
            nc.sync.dma_start(out=A6[:], in_=a6_ext[:])
            nc.vector.memset(S_chain[:, 0:1], 1.0)
            nc.vector.memset(pred_chain[:, 0:1], 0.0)

            span_ps = {}

            def dve_cluster(s_lo, ns):
                # reduce/scan/affine for strips [s_lo, s_lo+ns) of one span
                k, sq_lo = divmod(s_lo, 4)
                vv = span_ps[k][:, sq_lo * 96:(sq_lo + ns) * 96].rearrange(
                    "p (s c b x) -> p s c b x", s=ns, c=2, b=8, x=6
                )
                c_lo, nch = 2 * s_lo, 2 * ns
                csl = slice(c_lo, c_lo + nch)
                chview = lambda t: t[:, csl].rearrange(
                    "p (s c) -> p s c", s=ns, c=2
                )
                # per-chunk scale+mean in ONE reduce: permute the view so
                # the block dim is innermost and the (w,m) pair survives
                vp = span_ps[k][:, sq_lo * 96:(sq_lo + ns) * 96].rearrange(
                    "p (s c b x) -> p s c x b", s=ns, c=2, b=8, x=6
                )
                pm = pm_buf[:, 2 * c_lo:2 * (c_lo + nch)].rearrange(
                    "p (s c x) -> p s c x", s=ns, c=2, x=2
                )
                nc.vector.tensor_reduce(
                    out=pm,
                    in_=vp[:, :, :, 4:6, :],
                    axis=mybir.AxisListType.X,
                    op=ADD,
                )
                p_v = pm_buf[:, 2 * c_lo:2 * (c_lo + nch)].rearrange(
                    "p (c x) -> p c x", x=2
                )[:, :, 0:1].squeeze(2)
                m_v = pm_buf[:, 2 * c_lo:2 * (c_lo + nch)].rearrange(
                    "p (c x) -> p c x", x=2
                )[:, :, 1:2].squeeze(2)
                # scan cluster: S cumprod, tau = S*m, pred cumsum
                nc.vector.tensor_tensor_scan(
                    out=S_chain[:, c_lo + 1:c_lo + nch + 1],
                    data0=p_v,
                    data1=p_v,
                    initial=S_chain[:, c_lo:c_lo + 1],
                    op0=MUL,
                    op1=BYP,
                )
                nc.vector.tensor_tensor(
                    out=tau_buf[:, csl], in0=S_chain[:, csl], in1=m_v,
                    op=MUL,
                )
                nc.vector.tensor_tensor_scan(
                    out=pred_chain[:, c_lo + 1:c_lo + nch + 1],
                    data0=tau_buf[:, csl],
                    data1=tau_buf[:, csl],
                    initial=pred_chain[:, c_lo:c_lo + 1],
                    op0=ADD,
                    op1=BYP,
                )
                # affine: out = pred_c + S_c * y
                y5 = vv[:, :, :, :, 0:4]
                o5 = out_sb[:, 64 * s_lo:64 * (s_lo + ns)].rearrange(
                    "p (s c b x) -> p s c b x", s=ns, c=2, b=8, x=4
                )
                S_bc = (
                    chview(S_chain)
                    .unsqueeze(3).unsqueeze(4).broadcast_to([128, ns, 2, 8, 4])
                )
                P_bc = (
                    chview(pred_chain)
                    .unsqueeze(3).unsqueeze(4).broadcast_to([128, ns, 2, 8, 4])
                )
                nc.vector.tensor_tensor(out=o5, in0=y5, in1=S_bc, op=MUL)
                nc.vector.tensor_tensor(out=o5, in0=o5, in1=P_bc, op=ADD)

            def store(t_lo, t_hi):
                nc.sync.dma_start(
                    out=out_ext[:, t_lo:t_hi], in_=out_sb[:, t_lo:t_hi]
                )

            for s in range(NSTRIP):
                k, sq = divmod(s, 4)
                if sq == 0:
                    span_ps[k] = pspool.tile([128, 512], F32, name="ps", tag="ps")
                ps = span_ps[k]
                xv = xh[k]
                for blk in range(16):
                    nc.tensor.matmul(
                        ps[:, sq * 96 + 6 * blk:sq * 96 + 6 * blk + 6],
                        xv[:, sq * SF + blk * 128:sq * SF + (blk + 1) * 128],
                        A6[:],
                        start=True,
                        stop=True,
                    )
                if k < NQUAD - 1:
                    if sq == 3:
                        dve_cluster(4 * k, 4)
                        span_ps.pop(k)
                        store(256 * k, 256 * (k + 1))
                else:
                    # last span in half-span steps: strips 28-29 process and
                    # store while 30-31 still stream in
                    if sq == 1:
                        dve_cluster(s - 1, 2)
                        store(64 * (s - 1), 64 * (s + 1))
                    elif sq == 3:
                        dve_cluster(s - 1, 2)
                        span_ps.pop(k)
                        store(64 * (s - 1), 64 * (s + 1))

    nc.compile()
    return nc


def make_in_maps(inputs):
    x = np.asarray(inputs["x"], dtype=np.float32)
    # [c,b, quad,sq,blk,t'',n] -> [c, quad, t'', n, sq, blk, b], fp16
    xt = np.ascontiguousarray(
        x.astype(np.float16)
        .reshape(NCORES, BS, NQUAD, 4, 16, 4, 32)
        .transpose(0, 2, 5, 6, 3, 4, 1)
    ).reshape(NCORES, NQUAD * 128, 4 * SF)
    qb = np.asarray(inputs["quant_bins"], dtype=np.float32).reshape(NB)
    cs = np.asarray(inputs["change_scales"], dtype=np.float32).reshape(NB)
    a6 = np.zeros((128, 6), dtype=np.float16)
    for tp in range(4):
        sl = slice(32 * tp, 32 * tp + 32)
        a6[sl, tp] = qb
        a6[sl, 4] = cs / ADAPT
        a6[sl, 5] = qb / ADAPT
    return [
        {"x": xt[i], "a6": a6}
        for i in range(NCORES)
    ]


def gather_out(res):
    out = np.concatenate([res.results[i]["out"] for i in range(NCORES)], axis=0)
    return out.astype(np.float32)


def kernel(x, quant_bins, change_scales):
    global _cached_nc
    if _cached_nc is None:
        _cached_nc = build_kernel()
    nc = _cached_nc

    in_maps = make_in_maps(
        {"x": x, "quant_bins": quant_bins, "change_scales": change_scales}
    )
    res = run_bass_kernel_spmd(nc, in_maps, core_ids=list(range(NCORES)))
    return gather_out(res)


if __name__ == "__main__":
    rng = np.random.default_rng(0)
    x = rng.standard_normal((B, T, NB)).astype(np.float32)
    qb = rng.standard_normal((NB,)).astype(np.float32)
    cs = rng.uniform(0.9, 1.1, (NB, 1)).astype(np.float32)
    out = kernel(x=x, quant_bins=qb, change_scales=cs)
    print("out", out.shape, out.dtype)
